# revision 2
# baseline (speedup 1.0000x reference)
"""Trainium2 Bass kernel for windowed multi-head attention (Pangu-style).

Math per window w (144 tokens, dim 192, 6 heads x 32):
  qkv = x @ w_qkv + b_qkv ; per head: S = (q*scale) @ k^T + bias[rel] ;
  masked softmax ; out = (softmax @ v) concat ; y = out @ w_out + b_out

Device strategy (per core, 120 windows, batches of 3):
  - x transposed on PE (via identity matmul) into xT batch tiles [193, 432]
    (row 192 = ones, folds biases into all projections as an extra K row).
  - qT/kT generated with weights stationary, batched N=432 -> float32r
    runs at 1 cyc/row.
  - scores^T computed 3 heads at a time: lhsT = stacked kT [96,72],
    rhs = zero-padded block-diagonal qT [96,432] -> [72, 3*144].
  - E = exp(scores^T) * M2 where M2 = exp(biasT)*maskT is precomputed on
    host (kills the bias add on device; mask folds multiplicatively).
  - attn@v with 3 heads sharing one rhs: lhsT = [ones | v_h1 | v_h2 | v_h3]
    -> diagonal blocks of [97, 432] hold per-head outT, row 0 = softmax
    denominators.
  - normalize: reciprocal of the denominator row, broadcast across
    partitions with a K=1 ones-column matmul into PSUM, then per-head
    multiplies into attnT [96, 144] (c-major), which feeds the output
    projection directly (weights as rhs, N=256 padded -> float32r fast).
  - all matmul operands ride float32r (1 cyc/row at N>=256 vs 4 for fp32);
    producers bitcast outputs to float32r to satisfy the BIR verifier.
  - engine budget per window: PE ~25 matmuls; ACT: 2 exps + psum
    evacuations; DVE: reciprocals, 6 norm muls, copies; Pool: the 2 big
    E*M2 multiplies; SP/ACT HWDGE rings: 1 DMA each for x/m2/out.
"""

import sys

sys.path.insert(0, "/opt/trn_rl_repo")

import numpy as np

DIM = 192
HEADS = 6
HD = 32
N = 144
NW = 960
NCORES = 8
WPC = NW // NCORES  # 120 windows per core
GB = 2  # head groups
GH = 3  # heads per group
NG = GH * N  # 432

_NC_CACHE = {}


def _host_tensors(x, mask, rel_index, w_qkv, b_qkv, w_out, b_out, bias_table):
    f32 = np.float32
    scale = f32(1.0) / f32(np.sqrt(HD))
    w = np.array(w_qkv, f32, copy=True)
    b = np.array(b_qkv, f32, copy=True)
    w[:, :DIM] *= scale
    b[:DIM] *= scale

    # [193, 384]: q (scaled) and k weights with bias as extra contraction row
    wqk = np.concatenate([w[:, : 2 * DIM], b[None, : 2 * DIM]], axis=0)

    # [193, 256]: per group g: cols 97g+32h'+d = v weights of head 3g+h',
    # col 97g+96 = ones generator (bias row 1); rest zero pad
    wv = np.zeros((DIM + 1, 256), f32)
    for g in range(GB):
        wv[DIM, 97 * g + 96] = 1.0
        for hh in range(GH):
            h = GH * g + hh
            c0 = 97 * g + 32 * hh
            wv[:DIM, c0 : c0 + 32] = w[:, 2 * DIM + 32 * h : 2 * DIM + 32 * h + 32]
            wv[DIM, c0 : c0 + 32] = b[2 * DIM + 32 * h : 2 * DIM + 32 * h + 32]

    # [193, 256]: w_out with bias row, zero padded to 256 cols for fp32r
    wo = np.zeros((DIM + 1, 256), f32)
    wo[:DIM, :DIM] = np.asarray(w_out, f32)
    wo[DIM, :DIM] = np.asarray(b_out, f32)

    id72 = np.eye(72, dtype=f32)
    ones1 = np.ones((1, NG), f32)

    # biasT[h, j, mm, n] = bias_table[rel_index[n, 72j+mm], h]
    tab = np.asarray(bias_table, f32)
    ri = np.asarray(rel_index)
    bias_nm = tab[ri]  # [n, m, H]
    B = np.ascontiguousarray(bias_nm.transpose(2, 1, 0)).reshape(HEADS, 2, 72, N)
    B2 = np.zeros((GB, 2, 72, NG), f32)
    for g in range(GB):
        for hh in range(GH):
            B2[g, :, :, hh * N : (hh + 1) * N] = B[GH * g + hh]
    expB = np.exp(B2)  # [2, 2, 72, 432]

    # maskT[w, j, mm, n] = mask[w, n, 72j+mm]
    mk = np.asarray(mask)
    mT = np.ascontiguousarray(mk.transpose(0, 2, 1)).reshape(NW, 2, 72, N)
    mT = mT.astype(f32)
    # M2[w, g, mm, 432j + 144hh + n] = expB[g, j, mm, 144hh+n] * mT[w, j, mm, n]
    M2 = np.empty((NW, GB, 72, 2 * NG), f32)
    for g in range(GB):
        for j in range(2):
            tiled = np.tile(mT[:, j], (1, 1, GH))  # [NW, 72, 432]
            M2[:, g, :, NG * j : NG * (j + 1)] = tiled * expB[g, j][None]
    # [w, mm, g*864 + c] layout: one DMA per window
    M2 = np.ascontiguousarray(M2.transpose(0, 2, 1, 3)).reshape(NW, 72, 4 * NG)
    return wqk, wv, wo, id72, ones1, M2


def build_nc(wpc=WPC, repeat=1):
    import concourse.bass as bass
    import concourse.mybir as mybir
    from concourse import bacc, tile
    from contextlib import ExitStack

    f32 = mybir.dt.float32
    f32r = mybir.dt.float32r
    Exp = mybir.ActivationFunctionType.Exp
    Copy = mybir.ActivationFunctionType.Copy
    MUL = mybir.AluOpType.mult

    assert wpc % 3 == 0
    NB = wpc // 3

    nc = bacc.Bacc("TRN2", target_bir_lowering=False, debug=False)
    x_s = nc.declare_dram_parameter("x_s", [wpc, N, DIM], f32, isOutput=False)
    m2_s = nc.declare_dram_parameter("m2_s", [wpc, 72, 4 * NG], f32, isOutput=False)
    wqk_d = nc.declare_dram_parameter("wqk", [DIM + 1, 2 * DIM], f32, isOutput=False)
    wv_d = nc.declare_dram_parameter("wv", [DIM + 1, 256], f32, isOutput=False)
    wo_d = nc.declare_dram_parameter("wo", [DIM + 1, 256], f32, isOutput=False)
    id72_d = nc.declare_dram_parameter("id72", [72, 72], f32, isOutput=False)
    ones1_d = nc.declare_dram_parameter("ones1", [1, NG], f32, isOutput=False)
    zeros_d = nc.declare_dram_parameter("zeros96", [96, 2 * NG], f32, isOutput=False)
    y_s = nc.declare_dram_parameter("y_s", [wpc, N, DIM], f32, isOutput=True)

    def r(ap):
        return ap.bitcast(f32r)

    with ExitStack() as ctx:
        tc = ctx.enter_context(tile.TileContext(nc))
        cpool = ctx.enter_context(tc.tile_pool(name="const", bufs=1))
        sb = ctx.enter_context(tc.tile_pool(name="sb", bufs=2))
        pp = ctx.enter_context(tc.tile_pool(name="pp", bufs=1, space="PSUM"))

        # ---- constants ----
        w0 = cpool.tile([96, 2 * DIM], f32)
        w1 = cpool.tile([97, 2 * DIM], f32)
        nc.sync.dma_start(out=r(w0[:]), in_=r(wqk_d[0:96, :]))
        nc.sync.dma_start(out=r(w1[:]), in_=r(wqk_d[96:193, :]))
        v0 = cpool.tile([96, 256], f32)
        v1 = cpool.tile([97, 256], f32)
        nc.sync.dma_start(out=r(v0[:]), in_=r(wv_d[0:96, :]))
        nc.sync.dma_start(out=r(v1[:]), in_=r(wv_d[96:193, :]))
        wo0 = cpool.tile([96, 256], f32)
        wo1 = cpool.tile([96, 256], f32)
        wob = cpool.tile([1, 256], f32)
        nc.sync.dma_start(out=r(wo0[:]), in_=r(wo_d[0:96, :]))
        nc.sync.dma_start(out=r(wo1[:]), in_=r(wo_d[96:192, :]))
        nc.sync.dma_start(out=r(wob[:]), in_=r(wo_d[192:193, :]))
        idt = cpool.tile([72, 72], f32)
        nc.sync.dma_start(out=idt[:], in_=id72_d[:])
        onest = cpool.tile([1, NG], f32)
        nc.sync.dma_start(out=r(onest[:]), in_=r(ones1_d[:]))
        ones96 = cpool.tile([1, 96], f32)
        nc.sync.dma_start(out=r(ones96[:]), in_=r(ones1_d[:, 0:96]))

        # persistent zero-padded block-diagonal qT tiles [96, 2*432], per parity
        qzB = [cpool.tile([96, 2 * NG], f32, name=f"qzB{p}") for p in range(2)]
        for p in range(2):
            nc.sync.dma_start(out=r(qzB[p][:]), in_=r(zeros_d[:]))

        if repeat > 1:
            ctx.enter_context(tc.For_i(0, repeat, 1))

        for b in range(NB):
            # ---- xT batch tiles [96/97, 432] ----
            xT0 = sb.tile([96, NG], f32, tag="xt0")
            xT1 = sb.tile([97, NG], f32, tag="xt1")
            nc.sync.dma_start(out=r(xT1[96:97, :]), in_=r(ones1_d[:]))
            for wi in range(3):
                w = 3 * b + wi
                xtp0 = pp.tile([96, 144], f32, tag="xtp", bufs=2)
                xtp1 = pp.tile([96, 144], f32, tag="xtp", bufs=2)
                # one DMA per window: [144,192] viewed as [72, 2, 192]
                xa = sb.tile([72, 2 * DIM], f32, tag="xa", bufs=3)
                nc.sync.dma_start(out=xa[:],
                                  in_=x_s[w].rearrange("(j p) d -> p j d", j=2))
                for j in range(2):
                    nc.tensor.transpose(xtp0[:, 72 * j : 72 * j + 72],
                                        xa[:, 192 * j : 192 * j + 96], idt[:])
                    nc.tensor.transpose(xtp1[:, 72 * j : 72 * j + 72],
                                        xa[:, 192 * j + 96 : 192 * j + 192], idt[:])
                nc.vector.tensor_copy(r(xT0[:, 144 * wi : 144 * wi + 144]), xtp0[:])
                nc.scalar.activation(r(xT1[0:96, 144 * wi : 144 * wi + 144]),
                                     xtp1[:], Copy)

            # ---- qT/kT batch: 4 chunks of 96 cols, N=432 ----
            qAll = sb.tile([96, 2 * NG], f32, tag="qAll")
            ksb = []
            for mi in range(4):
                ps = pp.tile([96, NG], f32, tag="xtp", bufs=2)
                nc.tensor.matmul(ps[:], r(w0[:, 96 * mi : 96 * mi + 96]), r(xT0[:]),
                                 start=True, stop=False)
                nc.tensor.matmul(ps[:], r(w1[:, 96 * mi : 96 * mi + 96]), r(xT1[:]),
                                 start=False, stop=True)
                if mi < 2:
                    nc.scalar.activation(r(qAll[:, NG * mi : NG * mi + NG]), ps[:],
                                         Copy)
                else:
                    t = sb.tile([96, NG], f32, tag=f"qk{mi}")
                    nc.scalar.activation(r(t[:]), ps[:], Copy)
                    ksb.append(t)

            for wi in range(3):
                w = 3 * b + wi
                par = w % 2
                # qz diagonal refresh: per head-row block, both groups at once
                for hh in range(GH):
                    dst = qzB[par][32 * hh : 32 * hh + 32, :].rearrange(
                        "p (g c) -> p g c", g=2)[:, :, 144 * hh : 144 * hh + 144]
                    srcv = qAll[32 * hh : 32 * hh + 32, :].rearrange(
                        "p (g c) -> p g c", g=2)[:, :, 144 * wi : 144 * wi + 144]
                    if hh == 1:
                        nc.scalar.activation(r(dst), srcv, Copy)
                    else:
                        nc.vector.tensor_copy(r(dst), srcv)

                # ---- v generation: token-major [72, 256] per half ----
                vsb = []
                for j in range(2):
                    pvt = pp.tile([72, 256], f32, tag="pv", bufs=1)
                    sl = slice(144 * wi + 72 * j, 144 * wi + 72 * j + 72)
                    nc.tensor.matmul(pvt[:], r(xT0[:, sl]), r(v0[:]),
                                     start=True, stop=False)
                    nc.tensor.matmul(pvt[:], r(xT1[:, sl]), r(v1[:]),
                                     start=False, stop=True)
                    vt = sb.tile([72, 194], f32, tag=f"v{j}")
                    nc.vector.tensor_copy(r(vt[:]), pvt[:, 0:194])
                    vsb.append(vt)

                # ---- scores + exp + M2; one m2 DMA per window ----
                m2t = sb.tile([72, 4 * NG], f32, tag="m2", bufs=3)
                nc.scalar.dma_start(out=m2t[:], in_=m2_s[w])
                Es = []
                for g in range(GB):
                    et = sb.tile([72, 2 * NG], f32, tag=f"e{g}")
                    sct = pp.tile([72, 1024], f32, tag="sc", bufs=1)
                    for j in range(2):
                        sl = slice(144 * wi + 72 * j, 144 * wi + 72 * j + 72)
                        nc.tensor.matmul(sct[:, 512 * j : 512 * j + NG],
                                         r(ksb[g][:, sl]),
                                         r(qzB[par][:, NG * g : NG * g + NG]),
                                         start=True, stop=True)
                    sct_v = sct[:].rearrange("p (b c) -> p b c", b=2)[:, :, 0:NG]
                    nc.scalar.activation(r(et[:]), sct_v, Exp)
                    nc.gpsimd.tensor_tensor(r(et[:]), et[:],
                                      m2t[:, 2 * NG * g : 2 * NG * g + 2 * NG], MUL)
                    Es.append(et)

                # ---- attn @ v (3 heads share rhs; diagonal blocks valid) ----
                avs = []
                for g in range(GB):
                    avt = pp.tile([97, NG], f32, tag="av", bufs=2)
                    for j in range(2):
                        nc.tensor.matmul(avt[:], r(vsb[j][:, 97 * g : 97 * g + 97]),
                                         r(Es[g][:, NG * j : NG * j + NG]),
                                         start=(j == 0), stop=(j == 1))
                    avs.append(avt)

                # ---- normalize into attnT [96, 432] (c-major, batch cols) ----
                attnT = [sb.tile([96, 144], f32, tag=f"at{g}", name=f"at{g}_{b}_{wi}",
                                 bufs=3) for g in range(GB)]
                rcb = sb.tile([1, 2 * NG], f32, tag="rcb", bufs=3)
                with nc.allow_low_precision("fp32r rounding of denominators"):
                    for g in range(GB):
                        nc.vector.reciprocal(r(rcb[:, NG * g : NG * g + NG]),
                                             avs[g][96:97, :])
                asb, rbs = [], []
                for g in range(GB):
                    a = sb.tile([96, NG], f32, tag=f"asb{g}", bufs=2)
                    if g == 0:
                        nc.vector.tensor_copy(a[:], avs[g][0:96, :])
                    else:
                        nc.scalar.activation(a[:], avs[g][0:96, :], Copy)
                    asb.append(a)
                    rbt = pp.tile([96, NG], f32, tag="av", bufs=2)
                    nc.tensor.matmul(rbt[:], r(ones96[:]),
                                     r(rcb[:, NG * g : NG * g + NG]),
                                     start=True, stop=True)
                    rbs.append(rbt)
                for g in range(GB):
                    for hh in range(GH):
                        nc.vector.tensor_tensor(
                            r(attnT[g][32 * hh : 32 * hh + 32, :]),
                            asb[g][32 * hh : 32 * hh + 32, 144 * hh : 144 * hh + 144],
                            rbs[g][32 * hh : 32 * hh + 32, 144 * hh : 144 * hh + 144],
                            MUL)

                # ---- output projection, token-major, N=256 padded ----
                ot = sb.tile([72, 2 * DIM], f32, tag="ot", bufs=3)
                for j in range(2):
                    pft = pp.tile([72, 256], f32, tag="pf", bufs=1)
                    slw = slice(72 * j, 72 * j + 72)
                    sl = slice(144 * wi + 72 * j, 144 * wi + 72 * j + 72)
                    nc.tensor.matmul(pft[:], r(attnT[0][:, slw]), r(wo0[:]),
                                     start=True, stop=False)
                    nc.tensor.matmul(pft[:], r(attnT[1][:, slw]), r(wo1[:]),
                                     start=False, stop=False)
                    nc.tensor.matmul(pft[:], r(onest[:, sl]), r(wob[:]),
                                     start=False, stop=True)
                    if j == 0:
                        nc.vector.tensor_copy(ot[:, 0:DIM], pft[:, 0:DIM])
                    else:
                        nc.scalar.activation(ot[:, DIM : 2 * DIM],
                                             pft[:, 0:DIM], Copy)
                nc.scalar.dma_start(out=y_s[w].rearrange("(j p) d -> p j d", j=2),
                                  in_=ot[:])

    nc.compile()
    return nc


def make_in_maps(x, mask, rel_index, w_qkv, b_qkv, w_out, b_out, bias_table):
    wqk, wv, wo, id72, ones1, M2 = _host_tensors(
        x, mask, rel_index, w_qkv, b_qkv, w_out, b_out, bias_table)
    xf = np.asarray(x, np.float32)
    in_maps = []
    for c in range(NCORES):
        sl = slice(WPC * c, WPC * (c + 1))
        in_maps.append({
            "x_s": np.ascontiguousarray(xf[sl]),
            "m2_s": np.ascontiguousarray(M2[sl]),
            "wqk": wqk, "wv": wv, "wo": wo,
            "id72": id72, "ones1": ones1,
            "zeros96": np.zeros((96, 2 * NG), np.float32),
        })
    return in_maps


def kernel(x, mask, rel_index, w_qkv, b_qkv, w_out, b_out, bias_table):
    if WPC not in _NC_CACHE:
        _NC_CACHE[WPC] = build_nc(WPC)
    nc = _NC_CACHE[WPC]
    in_maps = make_in_maps(x, mask, rel_index, w_qkv, b_qkv, w_out, b_out,
                           bias_table)

    from concourse.bass_utils import run_bass_kernel_spmd
    import os
    trace = bool(os.environ.get("PANGU_TRACE"))
    kwargs = {}
    if trace:
        kwargs["trace"] = True
        if os.environ.get("PANGU_TRACE_DIR"):
            kwargs["tmpdir"] = os.environ["PANGU_TRACE_DIR"]
    res = run_bass_kernel_spmd(nc, in_maps, list(range(NCORES)), **kwargs)
    global LAST_EXEC_NS, LAST_RESULTS
    LAST_EXEC_NS = res.exec_time_ns
    LAST_RESULTS = res
    out = np.concatenate([res.results[c]["y_s"] for c in range(NCORES)], axis=0)
    return out.astype(np.float32)

